# revision 1
# baseline (speedup 1.0000x reference)
"""DTCWT inverse (qshift, single level) as a Bass/Tile kernel for TRN2.

Per-core computation, per (channel) slice:  Y = Ccat @ Xcat @ Rcat
with Xcat = [[Yl, hl], [lh, hh]] (c2q quadrants), Ccat/Rcat static banded
synthesis matrices. Implemented as two matmul stages that both consume
natural-layout data as the stationary operand:
    Tt = Xcat^T @ Ccat^T   (mm1, data stationary, statics moving)
    Y  = Tt^T  @ Rcat      (mm2)
Foldings (all host-side, into the static matrices):
  - quadrant row order:  D_E rows = [even; odd] (rho), D_O rows = [odd; even]
  - column order pi = [even cols; odd cols] per 128-block  -> Rcat rows
  - c2q 1/sqrt(2) scale -> quadrant statics
c2q itself is 2 scalar_tensor_tensor ops per quadrant (per-partition sign
vector), all operands partition-aligned at 0.
"""
import numpy as np

import concourse.bacc as bacc
import concourse.tile as tile
from concourse import mybir

F32 = mybir.dt.float32
F32R = mybir.dt.float32r

# ---------------- host-side static matrix construction ----------------

_H0A = np.array([0.0351638365171441, 0.0, -0.0883294244510729,
                 0.233890320607236, 0.760272369066126, 0.587518297723561,
                 0.0, -0.114301837144249, 0.0, 0.0], dtype=np.float64)
_H0B = _H0A[::-1].copy()
_ALT = (-1.0) ** np.arange(10)
_H1A = _H0B * _ALT
_H1B = _H1A[::-1].copy()
G0A, G0B, G1A, G1B = _H0B, _H0A, _H1B, _H1A

RHO_E = np.concatenate([np.arange(0, 128, 2), np.arange(1, 128, 2)])  # [even;odd]
RHO_O = np.concatenate([np.arange(1, 128, 2), np.arange(0, 128, 2)])  # [odd;even]
PI = RHO_E  # column order: even cols first


def _reflect(x, minx, maxx):
    x = np.asarray(x, dtype=np.float64)
    rng = maxx - minx
    rng2 = 2.0 * rng
    mod = np.fmod(x - minx, rng2)
    normed = np.where(mod < 0, mod + rng2, mod)
    return (np.where(normed >= rng, rng2 - normed, normed) + minx).astype(np.int64)


def _colifilt_matrix(ha, hb, r=128):
    """C (2r x r) with colifilt(X) = C @ X."""
    m = ha.shape[0]
    m2 = m // 2
    xe = _reflect(np.arange(-m2, r + m2), -0.5, r - 0.5)
    t = np.arange(2, r + m - 1, 2)
    if float(np.sum(ha * hb)) > 0:
        ta, tb = t, t - 1
    else:
        ta, tb = t - 1, t
    r2 = r // 2
    hao, hae = ha[0::2], ha[1::2]
    hbo, hbe = hb[0::2], hb[1::2]

    def vconv_mat(sel_idx, h):
        hf = h[::-1]
        M = np.zeros((r2, r), dtype=np.float64)
        for i in range(r2):
            for k in range(m2):
                M[i, sel_idx[i + k]] += hf[k]
        return M

    C = np.zeros((2 * r, r), dtype=np.float64)
    C[0::4] = vconv_mat(xe[tb], hao)
    C[1::4] = vconv_mat(xe[ta], hbo)
    C[2::4] = vconv_mat(xe[tb], hae)
    C[3::4] = vconv_mat(xe[ta], hbe)
    return C


def build_statics():
    """STAT1 (128 x 1280) = [S_TL | S_TR_E | S_TR_O | S_BL_E | S_BL_O
                             | ... wait: packed as 5 blocks? see below]
    Layout: [S_TL (256) | S_C0_E (256) | S_C0_O (256) | S_C1_E (256) | S_C1_O (256)]
      S_TL   = C0^T (natural rows)                       -- for the TL matmul
      S_C0_E = s * C0^T rows rho_E                       -- TR (hl) even cols
      S_C0_O = s * C0^T rows rho_O                       -- TR odd cols
      S_C1_E = s * C1^T rows rho_E                       -- BL (lh) / BR (hh) even
      S_C1_O = s * C1^T rows rho_O                       -- BL / BR odd
    STAT2 (128 x 512) = [R_lo' | R_hi'] with rows pi-permuted.
    SIGNS (128 x 2): col0 = [+1]*64+[-1]*64, col1 = [-1]*64+[+1]*64.
    """
    C0 = _colifilt_matrix(G0B, G0A)
    C1 = _colifilt_matrix(G1B, G1A)
    s = 1.0 / np.sqrt(2.0)
    # partition p of a band tile holds row r=p//2 of (real if p even else
    # imag); D_E row semantics are then the natural quadrant rows, D_O rows
    # are pair-swapped.
    swap = np.arange(128) ^ 1
    S_TL = C0.T
    S_C0_E = (s * C0).T
    S_C0_O = (s * C0[:, swap]).T
    S_C1_E = (s * C1).T
    S_C1_O = (s * C1[:, swap]).T
    STAT1 = np.concatenate([S_TL, S_C0_E, S_C0_O, S_C1_E, S_C1_O],
                           axis=1).astype(np.float32)
    R_lo = C0.T[PI]   # rows = Xcat cols, pi-permuted
    R_hi = C1.T[PI]
    STAT2 = np.concatenate([R_lo, R_hi], axis=1).astype(np.float32)
    SIGNS = np.zeros((128, 2), dtype=np.float32)
    SIGNS[0::2, 0] = 1.0    # x1 = +w2r + w1r   (even p = real rows)
    SIGNS[1::2, 0] = -1.0   # x3 = -w2i + w1i   (odd p = imag rows)
    SIGNS[0::2, 1] = -1.0   # x4 = -w1r + w2r
    SIGNS[1::2, 1] = 1.0    # x2 = +w1i + w2i
    return (np.ascontiguousarray(STAT1), np.ascontiguousarray(STAT2),
            np.ascontiguousarray(SIGNS))


# ---------------- device kernel ----------------

QUADS = [("hl", 2, 3, "C0"), ("lh", 0, 5, "C1"), ("hh", 1, 4, "C1")]


def build_kernel(n_ch=64, G=8, n_cores=8, merged_tl=False, debug_taps=False):
    """Build the per-core Bass module. Each core processes n_ch slices."""
    nc = bacc.Bacc("TRN2", target_bir_lowering=False, debug=False,
                   num_devices=n_cores)
    Yl = nc.dram_tensor("Yl", [n_ch, 128, 128], F32R, kind="ExternalInput").ap()
    YH = nc.dram_tensor("YH", [n_ch, 6, 128, 64], F32R, kind="ExternalInput").ap()
    ST1 = nc.dram_tensor("STAT1", [128, 1280], F32R, kind="ExternalInput").ap()
    ST2 = nc.dram_tensor("STAT2", [128, 512], F32R, kind="ExternalInput").ap()
    SGN = nc.dram_tensor("SIGNS", [128, 2], F32R, kind="ExternalInput").ap()
    OUT = nc.dram_tensor("Y", [n_ch, 256, 256], F32, kind="ExternalOutput").ap()

    assert n_ch % G == 0
    with tile.TileContext(nc) as tc:
        with (
            tc.tile_pool(name="const", bufs=1) as const,
            tc.tile_pool(name="inp", bufs=2) as inp,
            tc.tile_pool(name="quad", bufs=2) as quad,
            tc.tile_pool(name="tt", bufs=3) as ttp,
            tc.tile_pool(name="yout", bufs=2) as yp,
            tc.tile_pool(name="psum", bufs=3, space="PSUM") as pp,
            tc.tile_pool(name="psumy", bufs=2, space="PSUM") as ppy,
        ):
            s1 = const.tile([128, 1280], F32R)
            nc.sync.dma_start(s1[:], ST1[:])
            s2 = const.tile([128, 512], F32R)
            nc.sync.dma_start(s2[:], ST2[:])
            sgn = const.tile([128, 2], F32R)
            nc.sync.dma_start(sgn[:], SGN[:])

            # static rhs blocks
            S_TL = s1[:, 0:256]
            S_E = {"C0": s1[:, 256:512], "C1": s1[:, 768:1024]}
            S_O = {"C0": s1[:, 512:768], "C1": s1[:, 1024:1280]}
            R_lo = s2[:, 0:256]
            R_hi = s2[:, 256:512]
            s_a = sgn[:, 0:1]
            s_b = sgn[:, 1:2]


            def load_group(g0):
                TL = inp.tile([128, 128 * G], F32R, tag="TL")
                nc.sync.dma_start(
                    TL.rearrange("p (g c) -> p g c", g=G),
                    Yl[g0:g0 + G].rearrange("g p c -> p g c"),
                )
                bts = {}
                for qname, b1, b2, cmat in QUADS:
                    bt = inp.tile([128, 128 * G], F32R, tag=f"bt_{qname}")
                    btv = bt.rearrange("p (g b c) -> p g b c", g=G, b=2)
                    for bi, b in ((0, b1), (1, b2)):
                        nc.sync.dma_start(
                            btv[:, :, bi],
                            YH[g0:g0 + G, b].rearrange("g p c -> p g c"),
                        )
                    bts[qname] = btv
                return TL, bts

            def prep_group(state):
                TL, bts = state
                # odd Yl columns, packed contiguous (even cols are read from
                # TL with an even-offset stride-2 weight AP, which is legal)
                TL_O = inp.tile([128, 64 * G], F32R, tag="TL_O")
                nc.gpsimd.tensor_copy(
                    TL_O.rearrange("p (g j) -> p g j", g=G),
                    TL.rearrange("p (g j two) -> p g j two", g=G, two=2)[:, :, :, 1],
                )
                qt = {}
                for qname, b1, b2, cmat in QUADS:
                    btv = bts[qname]
                    B1 = btv[:, :, 0]   # rows: [w1r/w1i interleaved]
                    B2 = btv[:, :, 1]
                    D_E = quad.tile([128, 64 * G], F32R, tag=f"q_{qname}_E")
                    D_O = quad.tile([128, 64 * G], F32R, tag=f"q_{qname}_O")
                    qt[qname] = (D_E, D_O)
                    dev = D_E.rearrange("p (g c) -> p g c", g=G)
                    dov = D_O.rearrange("p (g c) -> p g c", g=G)
                    # D_E: even p: x1 = w2r + w1r ; odd p: x3 = -w2i + w1i
                    nc.vector.scalar_tensor_tensor(
                        dev, B2, s_a, B1,
                        op0=mybir.AluOpType.mult, op1=mybir.AluOpType.add)
                    # D_O: even p: x4 = -w1r + w2r ; odd p: x2 = w1i + w2i
                    nc.vector.scalar_tensor_tensor(
                        dov, B1, s_b, B2,
                        op0=mybir.AluOpType.mult, op1=mybir.AluOpType.add)
                return TL, TL_O, qt
            def process_group(g0, state, mid_emit=None):
                TL, TL_O, qt = state
                YB = yp.tile([128, 512 * G], F32, tag="yb")
                for ci in range(G):
                    if ci == 3 and mid_emit is not None:
                        mid_emit()
                    qs = slice(ci * 64, (ci + 1) * 64)
                    # fp32r matmuls cannot target PSUM partition base 64, so
                    # E/O halves go to free-dim halves of a 64p region; one
                    # two-bank tile per slice (bank0 = tt0, bank1 = tt1).
                    ttf = pp.tile([128, 1024], F32, tag="ttb")
                    tt0 = ttf[0:64, 0:512]
                    tt1 = ttf[0:64, 512:1024]
                    tl_even = TL[:, ci * 128:(ci + 1) * 128].rearrange(
                        "p (j two) -> p j two", two=2)[:, :, 0]
                    tl_odd = TL_O[:, qs]
                    # ONE start=True per PSUM bank: start marks the whole
                    # bank pending-zero, later matmuls accumulate anywhere.
                    nc.tensor.matmul(tt0[:, 0:256], tl_even, S_TL,
                                     start=True, stop=False, skip_group_check=True)
                    nc.tensor.matmul(tt0[:, 256:512], tl_odd, S_TL,
                                     start=False, stop=False, skip_group_check=True)
                    lhE, lhO = qt["lh"]
                    nc.tensor.matmul(tt0[:, 0:256], lhE[:, qs], S_E["C1"],
                                     start=False, stop=False, skip_group_check=True)
                    nc.tensor.matmul(tt0[:, 256:512], lhO[:, qs], S_O["C1"],
                                     start=False, stop=True, skip_group_check=True)
                    hlE, hlO = qt["hl"]
                    hhE, hhO = qt["hh"]
                    nc.tensor.matmul(tt1[:, 0:256], hlE[:, qs], S_E["C0"],
                                     start=True, stop=False, skip_group_check=True)
                    nc.tensor.matmul(tt1[:, 0:256], hhE[:, qs], S_E["C1"],
                                     start=False, stop=False, skip_group_check=True)
                    nc.tensor.matmul(tt1[:, 256:512], hlO[:, qs], S_O["C0"],
                                     start=False, stop=False, skip_group_check=True)
                    nc.tensor.matmul(tt1[:, 256:512], hhO[:, qs], S_O["C1"],
                                     start=False, stop=True, skip_group_check=True)

                    # TTS = [tt0s | tt1s] in one tile; E halves -> p0:64,
                    # O halves -> p64:128, each as one (64,512) copy
                    tts = ttp.tile([128, 512], F32R, tag="tts")
                    ttfv = ttf[0:64].rearrange("p (b eo n) -> p b eo n", b=2, eo=2)
                    ttsv = tts.rearrange("p (b n) -> p b n", b=2)
                    nc.scalar.copy(ttsv[0:64], ttfv[:, :, 0])
                    nc.vector.tensor_copy(ttsv[64:128], ttfv[:, :, 1])
                    tt0s = tts[:, 0:256]
                    tt1s = tts[:, 256:512]

                    ypb = ppy.tile([128, 512], F32, tag="ypb")
                    yp0 = ypb[:, 0:256]
                    yp1 = ypb[:, 256:512]
                    nc.tensor.matmul(yp0[:], tt0s[:, 0:128], R_lo,
                                     start=True, stop=False, skip_group_check=True)
                    nc.tensor.matmul(yp0[:], tt1s[:, 0:128], R_hi,
                                     start=False, stop=False, skip_group_check=True)
                    nc.tensor.matmul(yp1[:], tt0s[:, 128:256], R_lo,
                                     start=False, stop=False, skip_group_check=True)
                    nc.tensor.matmul(yp1[:], tt1s[:, 128:256], R_hi,
                                     start=False, stop=True, skip_group_check=True)

                    ocs = slice(ci * 512, (ci + 1) * 512)
                    nc.scalar.copy(YB[:, ocs], ypb[:])

                    if ci % 2 == 1:
                        c0 = g0 + ci - 1
                        fs = (ci - 1) * 512
                        # OUT[c, 0:128] <- YB slice [0:256]; OUT[c,128:256] <- [256:512]
                        nc.gpsimd.dma_start(
                            OUT[c0:c0 + 2].rearrange("g (h p) c -> p g h c", h=2),
                            YB[:, fs:fs + 1024].rearrange(
                                "p (g h c) -> p g h c", g=2, h=2),
                        )

            # software pipeline: emit loads+c2q of group g+1 before the
            # matmul/copy stream of group g
            groups = list(range(0, n_ch, G))
            state = prep_group(load_group(groups[0]))
            next_raw = [None]
            for idx, g0 in enumerate(groups):
                prepped = [None]
                if idx + 1 < len(groups):
                    next_raw[0] = load_group(groups[idx + 1])

                    def mid_emit(nr=next_raw, pr=prepped):
                        pr[0] = prep_group(nr[0])
                    process_group(g0, state, mid_emit)
                    state = prepped[0]
                else:
                    process_group(g0, state)

    nc.compile()
    return nc




# ---------------- host wrapper: shard, run on 8 cores, gather ----------------

_CACHED = {}


def _get_compiled():
    if "nc" not in _CACHED:
        _CACHED["nc"] = build_kernel(n_ch=64, G=8, n_cores=8)
        _CACHED["stats"] = build_statics()
    return _CACHED["nc"], _CACHED["stats"]


def _make_yh(Yhr, Yhi):
    """[C,6,64,64] x2 (fp32) -> [C,6,128,64] with real/imag row-interleave."""
    st = np.stack([Yhr, Yhi], axis=-2)          # [C,6,64,2,64]
    return np.ascontiguousarray(st.reshape(st.shape[0], 6, 128, 64))


def kernel(Yl, Yhr, Yhi):
    """Inverse DTCWT (qshift) level. Yl (8,64,128,128) f32,
    Yhr/Yhi (8,64,6,64,64) f32 -> (8,64,256,256) f32.
    Data-parallel over the batch dim: one batch element per NeuronCore."""
    from concourse.bass_utils import run_bass_kernel_spmd

    Yl = np.ascontiguousarray(np.asarray(Yl, dtype=np.float32))
    Yhr = np.asarray(Yhr, dtype=np.float32)
    Yhi = np.asarray(Yhi, dtype=np.float32)
    B = Yl.shape[0]
    assert B == 8, f"expected batch 8, got {B}"

    nc, (STAT1, STAT2, SIGNS) = _get_compiled()
    in_maps = []
    for b in range(B):
        in_maps.append({
            "Yl": np.ascontiguousarray(Yl[b]),
            "YH": _make_yh(Yhr[b], Yhi[b]),
            "STAT1": STAT1,
            "STAT2": STAT2,
            "SIGNS": SIGNS,
        })
    res = run_bass_kernel_spmd(nc, in_maps, core_ids=list(range(B)))
    out = np.stack([res.results[b]["Y"] for b in range(B)])
    return out.astype(np.float32)



# revision 3
# speedup vs baseline: 1.4478x; 1.4478x over previous
"""DTCWT inverse (qshift, single level) as a Bass/Tile kernel for TRN2.

Factorization (column filter first):
    out = (C0·Yl + C1·lh)·C0^T + (C0·hl + C1·hh)·C1^T
with C0/C1 the 256x128 banded synthesis (colifilt) matrices; the rowifilt
matrices are identical, so the same SBUF statics serve both stages.

Per slice (128x128 images), two PE stages of 4 matmuls each, all with
256 moving rows:
    stage 1:  z1^T = Yl^T-contract:  z1^T[c,i] = sum_r Yl[r,c]·C0T[r,i] + ...
              A = image (stationary, natural layout), B = C*T (moving)
              -> psum [128c, 512] = [z1^T | z2^T]
    stage 2:  out[i-tile] = zs-half^T @ C*T
              A = zs[:, tile] (stationary), B = C0T/C1T (moving)
              -> psum [128i, 512] = [rows 0:128 | rows 128:256]

Everything is bf16 (inputs, statics, mid-stage, output) with f32 PSUM
accumulation; validated rel err ~5.6e-3 vs the f32 reference.

c2q quadrants: quad = QA + QB where QA/QB are host-packed per-band images
(column-interleaved, signs baked in); one fused DVE add per group. The
1/sqrt(2) c2q scale is folded into the C0s/C1s statics used by stage 1
for the quadrant terms.

Host packs inputs partition-major so every DMA descriptor is a >=2KB
contiguous line; output rows are 512B contiguous bf16 writes.
"""
import numpy as np
import ml_dtypes

import concourse.bacc as bacc
import concourse.tile as tile
from concourse import mybir

F32 = mybir.dt.float32
BF16 = mybir.dt.bfloat16
NPBF16 = ml_dtypes.bfloat16

# quad -> (band1, band2): hl, lh, hh.  quad row filters: hl,lh,hh use
# (C0s, C1s, C1s) in stage 1 and pair with (C0, C0, C1)... see stage map.
QUAD_BANDS = ((2, 3), (0, 5), (1, 4))   # hl, lh, hh

# ---------------- host-side static matrix construction ----------------

_H0A = np.array([0.0351638365171441, 0.0, -0.0883294244510729,
                 0.233890320607236, 0.760272369066126, 0.587518297723561,
                 0.0, -0.114301837144249, 0.0, 0.0], dtype=np.float64)
_H0B = _H0A[::-1].copy()
_ALT = (-1.0) ** np.arange(10)
_H1A = _H0B * _ALT
_H1B = _H1A[::-1].copy()
G0A, G0B, G1A, G1B = _H0B, _H0A, _H1B, _H1A


def _reflect(x, minx, maxx):
    x = np.asarray(x, dtype=np.float64)
    rng = maxx - minx
    rng2 = 2.0 * rng
    mod = np.fmod(x - minx, rng2)
    normed = np.where(mod < 0, mod + rng2, mod)
    return (np.where(normed >= rng, rng2 - normed, normed) + minx).astype(np.int64)


def _colifilt_matrix(ha, hb, r=128):
    """C (2r x r) with colifilt(X) = C @ X."""
    m = ha.shape[0]
    m2 = m // 2
    xe = _reflect(np.arange(-m2, r + m2), -0.5, r - 0.5)
    t = np.arange(2, r + m - 1, 2)
    if float(np.sum(ha * hb)) > 0:
        ta, tb = t, t - 1
    else:
        ta, tb = t - 1, t
    r2 = r // 2
    hao, hae = ha[0::2], ha[1::2]
    hbo, hbe = hb[0::2], hb[1::2]

    def vconv_mat(sel_idx, h):
        hf = h[::-1]
        M = np.zeros((r2, r), dtype=np.float64)
        for i in range(r2):
            for k in range(m2):
                M[i, sel_idx[i + k]] += hf[k]
        return M

    C = np.zeros((2 * r, r), dtype=np.float64)
    C[0::4] = vconv_mat(xe[tb], hao)
    C[1::4] = vconv_mat(xe[ta], hbo)
    C[2::4] = vconv_mat(xe[tb], hae)
    C[3::4] = vconv_mat(xe[ta], hbe)
    return C


def build_statics():
    """CT [128, 1024] bf16 = [C0T | C1T | C0sT | C1sT], s = 1/sqrt(2)."""
    C0 = _colifilt_matrix(G0B, G0A)
    C1 = _colifilt_matrix(G1B, G1A)
    s = 1.0 / np.sqrt(2.0)
    CT = np.concatenate([C0.T, C1.T, (s * C0).T, (s * C1).T], axis=1)
    return np.ascontiguousarray(CT.astype(np.float32).astype(NPBF16))


# ---------------- device kernel ----------------


def build_kernel(n_ch=64, G=8, n_cores=8):
    nc = bacc.Bacc("TRN2", target_bir_lowering=False, debug=False,
                   num_devices=n_cores)
    YLT = nc.dram_tensor("YLT", [128, n_ch, 128], BF16, kind="ExternalInput").ap()
    QA = nc.dram_tensor("QA", [3, 128, n_ch, 128], BF16, kind="ExternalInput").ap()
    QB = nc.dram_tensor("QB", [3, 128, n_ch, 128], BF16, kind="ExternalInput").ap()
    CTD = nc.dram_tensor("CT", [128, 1024], BF16, kind="ExternalInput").ap()
    OUT = nc.dram_tensor("Y", [n_ch, 256, 256], BF16, kind="ExternalOutput").ap()

    assert n_ch % G == 0
    with tile.TileContext(nc) as tc:
        with (
            tc.tile_pool(name="const", bufs=1) as const,
            tc.tile_pool(name="inp", bufs=2) as inp,
            tc.tile_pool(name="quad", bufs=2) as quad,
            tc.tile_pool(name="zt", bufs=3) as ztp,
            tc.tile_pool(name="yout", bufs=2) as yp,
            tc.tile_pool(name="psz", bufs=2, space="PSUM") as pp,
            tc.tile_pool(name="psy", bufs=2, space="PSUM") as ppy,
        ):
            ct = const.tile([128, 1024], BF16)
            nc.sync.dma_start(ct[:], CTD[:])
            C0T = ct[:, 0:256]
            C1T = ct[:, 256:512]
            C0sT = ct[:, 512:768]
            C1sT = ct[:, 768:1024]

            def load_group(g0):
                TL = inp.tile([128, G * 128], BF16, tag="TL")
                nc.sync.dma_start(
                    TL.rearrange("p (g c) -> p g c", g=G),
                    YLT[:, g0:g0 + G],
                )
                qa = inp.tile([128, 3 * G * 128], BF16, tag="qa")
                qb = inp.tile([128, 3 * G * 128], BF16, tag="qb")
                nc.sync.dma_start(
                    qa.rearrange("p (q g c) -> p q g c", q=3, g=G),
                    QA[:, :, g0:g0 + G].rearrange("q p g c -> p q g c"),
                )
                nc.sync.dma_start(
                    qb.rearrange("p (q g c) -> p q g c", q=3, g=G),
                    QB[:, :, g0:g0 + G].rearrange("q p g c -> p q g c"),
                )
                return TL, qa, qb

            def prep_group(state):
                TL, qa, qb = state
                QD = quad.tile([128, 3 * G * 128], BF16, tag="qd")
                nc.vector.tensor_add(QD[:], qa[:], qb[:])
                return TL, QD

            def process_group(g0, state):
                TL, QD = state
                qv = QD.rearrange("p (q g c) -> p q g c", q=3, g=G)
                YB = yp.tile([128, G * 512], BF16, tag="yb")
                for ci in range(G):
                    cs = slice(ci * 128, (ci + 1) * 128)
                    hl = qv[:, 0, ci]
                    lh = qv[:, 1, ci]
                    hh = qv[:, 2, ci]
                    zp = pp.tile([128, 512], F32, tag="zp")
                    nc.tensor.matmul(zp[:, 0:256], TL[:, cs], C0T,
                                     start=True, stop=False, skip_group_check=True)
                    nc.tensor.matmul(zp[:, 0:256], lh, C1sT,
                                     start=False, stop=False, skip_group_check=True)
                    nc.tensor.matmul(zp[:, 256:512], hl, C0sT,
                                     start=False, stop=False, skip_group_check=True)
                    nc.tensor.matmul(zp[:, 256:512], hh, C1sT,
                                     start=False, stop=True, skip_group_check=True)

                    zs = ztp.tile([128, 512], BF16, tag="zs")
                    nc.scalar.copy(zs[:], zp[:])

                    op = ppy.tile([128, 512], F32, tag="op")
                    nc.tensor.matmul(op[:, 0:256], zs[:, 0:128], C0T,
                                     start=True, stop=False, skip_group_check=True)
                    nc.tensor.matmul(op[:, 0:256], zs[:, 256:384], C1T,
                                     start=False, stop=False, skip_group_check=True)
                    nc.tensor.matmul(op[:, 256:512], zs[:, 128:256], C0T,
                                     start=False, stop=False, skip_group_check=True)
                    nc.tensor.matmul(op[:, 256:512], zs[:, 384:512], C1T,
                                     start=False, stop=True, skip_group_check=True)

                    nc.vector.tensor_copy(YB[:, ci * 512:(ci + 1) * 512], op[:])

                nc.gpsimd.dma_start(
                    OUT[g0:g0 + G].rearrange("g (it p) w -> p g it w", it=2),
                    YB.rearrange("p (g it w) -> p g it w", g=G, it=2),
                )

            # software pipeline: load g+1 before computing g
            groups = list(range(0, n_ch, G))
            state = prep_group(load_group(groups[0]))
            for idx, g0 in enumerate(groups):
                if idx + 1 < len(groups):
                    nxt = load_group(groups[idx + 1])
                    process_group(g0, state)
                    state = prep_group(nxt)
                else:
                    process_group(g0, state)

    nc.compile()
    return nc


# ---------------- host wrapper: shard, run on 8 cores, gather ----------------

_CACHED = {}


def _get_compiled():
    if "nc" not in _CACHED:
        _CACHED["nc"] = build_kernel(n_ch=64, G=8, n_cores=8)
        _CACHED["ct"] = build_statics()
    return _CACHED["nc"], _CACHED["ct"]


def _make_in_maps(Yl, Yhr, Yhi, CT):
    """Per-core input packing (pure layout: transpose/interleave/sign)."""
    B = Yl.shape[0]
    # YLT: [B, 128(r), C, 128(c)]
    YLT = np.ascontiguousarray(
        Yl.transpose(0, 2, 1, 3)).astype(NPBF16)
    QA = np.zeros((B, 3, 128, 64, 128), dtype=np.float32)
    QB = np.zeros((B, 3, 128, 64, 128), dtype=np.float32)
    for q, (b1, b2) in enumerate(QUAD_BANDS):
        r1 = Yhr[:, :, b1].transpose(0, 2, 1, 3)   # [B, h, C, w]
        i1 = Yhi[:, :, b1].transpose(0, 2, 1, 3)
        r2 = Yhr[:, :, b2].transpose(0, 2, 1, 3)
        i2 = Yhi[:, :, b2].transpose(0, 2, 1, 3)
        QA[:, q, 0::2, :, 0::2] = r1
        QA[:, q, 0::2, :, 1::2] = i1
        QA[:, q, 1::2, :, 0::2] = i1
        QA[:, q, 1::2, :, 1::2] = -r1
        QB[:, q, 0::2, :, 0::2] = r2
        QB[:, q, 0::2, :, 1::2] = i2
        QB[:, q, 1::2, :, 0::2] = -i2
        QB[:, q, 1::2, :, 1::2] = r2
    QA = QA.astype(NPBF16)
    QB = QB.astype(NPBF16)
    in_maps = []
    for b in range(B):
        in_maps.append({
            "YLT": np.ascontiguousarray(YLT[b]),
            "QA": np.ascontiguousarray(QA[b]),
            "QB": np.ascontiguousarray(QB[b]),
            "CT": CT,
        })
    return in_maps


def kernel(Yl, Yhr, Yhi):
    """Inverse DTCWT (qshift) level. Yl (8,64,128,128) f32,
    Yhr/Yhi (8,64,6,64,64) f32 -> (8,64,256,256) f32.
    Data-parallel over the batch dim: one batch element per NeuronCore."""
    from concourse.bass_utils import run_bass_kernel_spmd

    Yl = np.asarray(Yl, dtype=np.float32)
    Yhr = np.asarray(Yhr, dtype=np.float32)
    Yhi = np.asarray(Yhi, dtype=np.float32)
    B = Yl.shape[0]
    assert B == 8, f"expected batch 8, got {B}"

    nc, CT = _get_compiled()
    in_maps = _make_in_maps(Yl, Yhr, Yhi, CT)
    res = run_bass_kernel_spmd(nc, in_maps, core_ids=list(range(B)))
    out = np.stack([np.asarray(res.results[b]["Y"]) for b in range(B)])
    return out.astype(np.float32)
